# revision 1
# baseline (speedup 1.0000x reference)
"""Multi-head attention (softmax over the QUERY axis) for Trainium2, 8 cores.

Reference computation (B=2, T=2048, E=1024, H=16, HD=64):
    q = split_heads(X @ Wq.T + bq); k = ...; v = ...
    scores = (q @ k^T) / sqrt(E), causally masked (key > query -> -inf)
    attn   = softmax(scores, axis=QUERY)      # <- normalizes over q, per key
    out    = attn @ v, merged heads

Sharding: core c handles batch c//4 and head group c%4 (4 heads = 256 output
dims).  No collectives needed.  Host pre-transposes X and the weight slices so
the device never transposes anything.

Device layout (per core).  Projection/scores matmul operands are declared
float32r (full PE rate, vs 4 cycles/row for plain fp32); the A*V operands
(P, V') are float16, which permits col tile_position packing (fp32r does
not pass the walrus ISA check for it):
    XT [1024,2048]  (e on partitions via 8 chunks of 128)
    QT/KT = W.T.T @ XT + b  ->  [256 d, 2048 t]   (2 partition tiles of 128)
    V     = XT.T @ WvT + bv ->  [2048 t, 256 d]   (16 t-tiles of 128)
    per (duo of 2 heads, k-tile of 128 keys):
        S^T[k, q] = K^T.T @ Q^T  (heads packed in the PE via row tile_position)
        mask diag block, P = exp(S^T/32) via ACT (PSUM->SBUF) + row-sum accum
        r = 1/sum, V' = V * r (per-key scaling replaces the softmax divide)
        O^T[d, q] += V'.T @ P   (fp16; the two heads' matmuls run in separate
        PE column groups concurrently, accumulating over k-tiles into four
        per-bank PSUM tiles; bank 0 is time-shared with projections/V)
    O^T [256, 2048] -> DRAM; host writes out[b, :, g*256:(g+1)*256] = O^T.T
"""

from contextlib import ExitStack

import numpy as np

import concourse.bacc as bacc
import concourse.mybir as mybir
import concourse.tile as tile
from concourse.bass_utils import run_bass_kernel_spmd

B, T, E, H = 2, 2048, 1024, 16
HD = 64
D2 = 256           # output dims per core (4 heads)
NKT = T // 128     # 16 k-tiles
F32 = mybir.dt.float32
F32R = mybir.dt.float32r
F16 = mybir.dt.float16
MDT = F32R     # dtype for all matmul operands (PE runs full speed on f32r)
EXP = mybir.ActivationFunctionType.Exp
AX = mybir.AxisListType.X
SCALE = 1.0 / 32.0  # 1/sqrt(E)
NEG = -1.0e30

_CACHE = {}


def _build_module():
    nc = bacc.Bacc("TRN2", target_bir_lowering=False, debug=False)

    xt_d = nc.dram_tensor("xt", [E, T], MDT, kind="ExternalInput")
    wqt_d = nc.dram_tensor("wqt", [E, D2], MDT, kind="ExternalInput")
    wkt_d = nc.dram_tensor("wkt", [E, D2], MDT, kind="ExternalInput")
    wvt_d = nc.dram_tensor("wvt", [E, D2], MDT, kind="ExternalInput")
    bqc_d = nc.dram_tensor("bqc", [128, 2], F32, kind="ExternalInput")
    bkc_d = nc.dram_tensor("bkc", [128, 2], F32, kind="ExternalInput")
    bvr_d = nc.dram_tensor("bvr", [1, D2], MDT, kind="ExternalInput")
    mask_d = nc.dram_tensor("mask", [128, 128], F32, kind="ExternalInput")
    ones_d = nc.dram_tensor("onesr", [1, 512], MDT, kind="ExternalInput")
    zr_d = nc.dram_tensor("zr", [128, 128], MDT, kind="ExternalInput")
    ot_d = nc.dram_tensor("ot", [D2, T], F32, kind="ExternalOutput")

    with tile.TileContext(nc) as tc:
        _body(tc, xt_d, wqt_d, wkt_d, wvt_d, bqc_d, bkc_d, bvr_d, mask_d,
              ones_d, zr_d, ot_d)
    nc.compile()
    return nc


def _body(tc, xt_d, wqt_d, wkt_d, wvt_d, bqc_d, bkc_d, bvr_d, mask_d,
          ones_d, zr_d, ot_d):
    nc = tc.nc

    with ExitStack() as ctx:
        const_pool = ctx.enter_context(tc.tile_pool(name="const", bufs=1))
        ones_t = const_pool.tile([1, 512], MDT)
        nc.sync.dma_start(ones_t[:], ones_d.ap())
        mask_t = const_pool.tile([128, 128], F32)
        nc.sync.dma_start(mask_t[:], mask_d.ap())
        bqc_t = const_pool.tile([128, 2], F32)
        nc.sync.dma_start(bqc_t[:], bqc_d.ap())
        bkc_t = const_pool.tile([128, 2], F32)
        nc.sync.dma_start(bkc_t[:], bkc_d.ap())
        bvr_t = const_pool.tile([1, D2], MDT)
        nc.sync.dma_start(bvr_t[:], bvr_d.ap())

        # V'-weights ping-pong tiles, fp16: cols [0:64]=vp_h0,
        # [64:128]=vp_h1.  The A*V matmuls run in fp16 so the two heads pack
        # into the PE concurrently via col tile_position (fp32r col-packing
        # fails walrus ISA checks; fp16 is the documented packing path).
        vp_pool = ctx.enter_context(tc.tile_pool(name="vp", bufs=1))
        vp_ab = []
        for i in range(2):
            vp = vp_pool.tile([128, 128], F16, name=f"vp{i}")
            vp_ab.append(vp)

        proj_pool = ctx.enter_context(tc.tile_pool(name="proj", bufs=1))
        qt_t = proj_pool.tile([128, 2 * T], MDT)   # [:, dt*T + t]
        kt_t = proj_pool.tile([128, 2 * T], MDT)
        v_t = proj_pool.tile([128, NKT * D2], F32)  # [:, tt*D2 + d]

        # Projections are issued on demand inside the attention loop, so duo
        # 1's projections fill PE gaps while duo 0's attention keeps ACT busy.
        # PSUM budget (8 banks): 3x [128,512] scores slots (exp pipeline
        # stays fed), 1x [128,512] projection/V slot, 4-bank O^T accumulator.
        with (
            tc.tile_pool(name="xt", bufs=1) as xt_pool,
            tc.tile_pool(name="w", bufs=1) as w_pool,
            tc.tile_pool(name="p", bufs=2) as p_pool,
            tc.tile_pool(name="stats", bufs=3) as st_pool,
            tc.tile_pool(name="osb", bufs=1) as osb_pool,
            tc.tile_pool(name="sc_ps", bufs=2, space="PSUM") as sc_pool,
            tc.tile_pool(name="ot_ps", bufs=1, space="PSUM") as ot_pool,
        ):
            # Warm the ACT exp table off the critical path (first real exp
            # otherwise pays the ~1.3us table load mid-pipeline).
            warm_t = st_pool.tile([1, 2], F32, name="warm")
            nc.scalar.activation(warm_t[:], mask_t[0:1, 0:2], EXP,
                                 bias=0.0, scale=SCALE)

            # DMA order: wq/wk chunk just before its xt chunk (so Q/K
            # projections complete right as the last xt chunk lands); wv is
            # only needed for V tiles, which trail — load it last.
            xt_t = xt_pool.tile([128, 8 * T], MDT)  # [:, ec*T + t]
            wq_t = w_pool.tile([128, 8 * D2], MDT)  # [:, ec*D2 + d]
            wk_t = w_pool.tile([128, 8 * D2], MDT)
            wv_t = w_pool.tile([128, 8 * D2], MDT)
            for ec in range(8):
                for w_sb, w_dr in ((wq_t, wqt_d), (wk_t, wkt_d)):
                    nc.sync.dma_start(
                        w_sb[:, ec * D2:(ec + 1) * D2],
                        w_dr.ap()[ec * 128:(ec + 1) * 128, :],
                    )
                nc.sync.dma_start(
                    xt_t[:, ec * T:(ec + 1) * T],
                    xt_d.ap()[ec * 128:(ec + 1) * 128, :],
                )
            for ec in range(8):
                nc.sync.dma_start(
                    wv_t[:, ec * D2:(ec + 1) * D2],
                    wvt_d.ap()[ec * 128:(ec + 1) * 128, :],
                )

            def emit_v_tile(tt):
                # V[tt]: [128 t, D2] = XT.T @ WvT + ones.T @ bv
                ps = ot_pool.tile([128, 512], F32, tag="ot0", name="ps_v")
                pv = ps[:, 0:D2]
                for ec in range(8):
                    nc.tensor.matmul(
                        pv,
                        lhsT=xt_t[:, ec * T + tt * 128:ec * T + tt * 128 + 128],
                        rhs=wv_t[:, ec * D2:(ec + 1) * D2],
                        start=(ec == 0),
                        stop=False,
                    )
                nc.tensor.matmul(
                    pv,
                    lhsT=ones_t[0:1, 0:128],
                    rhs=bvr_t[0:1, :],
                    start=False,
                    stop=True,
                )
                nc.vector.tensor_copy(v_t[:, tt * D2:(tt + 1) * D2], pv)

            def emit_qk_chunk(pduo, is_k, c, on_ot0=False):
                # one 512-wide QT/KT projection chunk for duo `pduo`.
                # Chunks that run while O^T bank 0 is idle (duo 1's, injected
                # into duo 0's late k-tiles; startup before the accumulator
                # exists) time-share the ot0 bank; chunks needed while ot0 is
                # live briefly borrow a scores slot instead.
                out_t, w_sb, b_sb = ((kt_t, wk_t, bkc_t) if is_k
                                     else (qt_t, wq_t, bqc_t))
                if on_ot0:
                    ps = ot_pool.tile([128, 512], F32, tag="ot0", name="ps_qk")
                else:
                    ps = sc_pool.tile([128, 512], F32, tag="sc", name="ps_qk")
                for ec in range(8):
                    nc.tensor.matmul(
                        ps[:],
                        lhsT=w_sb[:, ec * D2 + pduo * 128:
                                  ec * D2 + pduo * 128 + 128],
                        rhs=xt_t[:, ec * T + c * 512:ec * T + c * 512 + 512],
                        start=(ec == 0),
                        stop=(ec == 7),
                    )
                nc.vector.tensor_scalar_add(
                    out_t[:, pduo * T + c * 512:pduo * T + c * 512 + 512],
                    ps[:],
                    b_sb[:, pduo:pduo + 1],
                )

            emitted = set()

            def ensure_qk(pduo, is_k, c, on_ot0=False):
                if (pduo, is_k, c) not in emitted:
                    emitted.add((pduo, is_k, c))
                    emit_qk_chunk(pduo, is_k, c, on_ot0=on_ot0)

            for duo in range(2):
                # Projections are emitted on demand (first use by a scores
                # piece), so the exp pipeline starts as early as possible.
                # kt0's chunks are pre-emitted spread over BOTH psum pools so
                # they accumulate concurrently while X streams in; duo 1's
                # chunks are injected into duo 0's late k-tiles so they fill
                # PE slack while ACT stays busy.
                if duo == 0:
                    ensure_qk(0, False, 0)
                    ensure_qk(0, False, 1)
                    ensure_qk(0, True, 0, on_ot0=True)
                    ensure_qk(0, False, 2)
                    ensure_qk(0, False, 3)
                    for tt in range(4):
                        emit_v_tile(tt)
                    inject = {1: [(0, True, 1)],
                              4: [(0, True, 2)],
                              6: [(1, False, 0)],
                              7: [(1, True, 0)],
                              8: [(0, True, 3)],
                              9: [(1, False, 1)],
                              10: [(1, False, 2)],
                              11: [(1, False, 3), (1, True, 1)],
                              12: [(1, True, 2)],
                              13: [(1, True, 3)]}
                else:
                    inject = {}

                # ---- attention for this duo ----
                ot_bk = [ot_pool.tile([128, 512], F32, tag=f"ot{b}",
                                      name=f"ot{b}") for b in range(4)]
                ot_sb = osb_pool.tile([128, T], F32, tag="osb", name="ot_sb")
                for kt in range(NKT):
                    qlo = kt * 128
                    W = T - qlo
                    pieces = []
                    poff = 0
                    while poff < W:
                        pieces.append((poff, min(1024, W - poff)))
                        poff += 1024

                    sums_t = st_pool.tile([128, 4], F32, tag="sums", name="sums")
                    rinv_t = st_pool.tile([128, 2], F32, tag="rinv", name="rinv")

                    p_ts = []
                    for hh in range(2):
                        p_t = p_pool.tile([128, T], F16, tag=f"p{hh}",
                                          name=f"p{hh}")
                        p_ts.append(p_t)
                        d0 = 64 * hh
                        for pi, (poff, pw) in enumerate(pieces):
                            ensure_qk(duo, True, kt // 4)
                            for c in range((qlo + poff) // 512,
                                           (qlo + poff + pw - 1) // 512 + 1):
                                ensure_qk(duo, False, c)
                            sc = sc_pool.tile([128, 1024], F32, tag="sc",
                                              name="sc")
                            for co in range(0, pw, 512):
                                n = min(512, pw - co)
                                nc.tensor.matmul(
                                    sc[:, co:co + n],
                                    lhsT=kt_t[d0:d0 + 64,
                                              duo * T + qlo:
                                              duo * T + qlo + 128],
                                    rhs=qt_t[d0:d0 + 64,
                                             duo * T + qlo + poff + co:
                                             duo * T + qlo + poff + co + n],
                                    start=True,
                                    stop=True,
                                )
                            if poff == 0:
                                nc.vector.tensor_add(sc[:, 0:128], sc[:, 0:128],
                                                     mask_t[:])
                            nc.scalar.activation(
                                p_t[:, poff:poff + pw],
                                sc[:, 0:pw],
                                EXP,
                                bias=0.0,
                                scale=SCALE,
                                accum_out=sums_t[:, hh * 2 + pi:hh * 2 + pi + 1],
                            )

                    if duo == 0 and kt < NKT - 4:
                        emit_v_tile(kt + 4)
                    for args in inject.get(kt, ()):
                        pduo_i = args[0]
                        ensure_qk(*args, on_ot0=(pduo_i == 1))

                    # 1/sum; V' = V * r into the zero-padded weight tile
                    vp_t = vp_ab[kt % 2]
                    np_ = len(pieces)
                    for hh in range(2):
                        if np_ > 1:
                            rs_t = st_pool.tile([128, 1], F32, tag=f"rs{hh}",
                                                name=f"rs{hh}")
                            nc.vector.reduce_sum(rs_t[:],
                                                 sums_t[:, hh * 2:hh * 2 + np_],
                                                 axis=AX)
                        else:
                            rs_t = sums_t[:, hh * 2:hh * 2 + 1]
                        nc.vector.reciprocal(rinv_t[:, hh:hh + 1], rs_t[:])
                        dst = vp_t[:, 0:64] if hh == 0 else vp_t[:, 64:128]
                        nc.vector.tensor_scalar_mul(
                            dst,
                            v_t[:, kt * D2 + duo * 128 + 64 * hh:
                                kt * D2 + duo * 128 + 64 * hh + 64],
                            rinv_t[:, hh:hh + 1],
                        )

                    # O^T[:, q] += V'.T @ P, bank-aligned chunks of 512.
                    # Head 0 owns start= (first write of the bank), head 1
                    # owns stop= on the bank's last k-tile; finished banks are
                    # copied out immediately so the tail stays short.
                    c0 = qlo
                    while c0 < T:
                        bank = c0 // 512
                        c1 = min((bank + 1) * 512, T)
                        last_kt = min(4 * bank + 3, NKT - 1)
                        for hh in range(2):
                            nc.tensor.matmul(
                                ot_bk[bank][64 * hh:64 * hh + 64,
                                            c0 - bank * 512:c1 - bank * 512],
                                lhsT=vp_t[:, 64 * hh:64 * hh + 64],
                                rhs=p_ts[hh][:, c0 - qlo:c1 - qlo],
                                start=(kt == 0),
                                stop=(kt == last_kt),
                            )
                        if kt == last_kt:
                            nc.vector.tensor_copy(
                                ot_sb[:, bank * 512:bank * 512 + 512],
                                ot_bk[bank][:])
                            nc.sync.dma_start(
                                ot_d.ap()[duo * 128:(duo + 1) * 128,
                                          bank * 512:bank * 512 + 512],
                                ot_sb[:, bank * 512:bank * 512 + 512])
                        c0 = c1


def _get_module():
    if "nc" not in _CACHE:
        _CACHE["nc"] = _build_module()
    return _CACHE["nc"]


def _make_mask():
    k = np.arange(128)[:, None]
    q = np.arange(128)[None, :]
    return np.where(q >= k, 0.0, NEG).astype(np.float32)


def _make_in_maps(X, Wq, bq, Wk, bk, Wv, bv):
    X = np.asarray(X, np.float32)
    mask = _make_mask()
    ones = np.ones((1, 512), np.float32)
    zr = np.zeros((128, 128), np.float32)
    in_maps = []
    for c in range(8):
        b, g = divmod(c, 4)
        rows = slice(D2 * g, D2 * g + D2)
        in_maps.append({
            "xt": np.ascontiguousarray(X[b].T),
            "wqt": np.ascontiguousarray(np.asarray(Wq)[rows].T),
            "wkt": np.ascontiguousarray(np.asarray(Wk)[rows].T),
            "wvt": np.ascontiguousarray(np.asarray(Wv)[rows].T),
            "bqc": np.ascontiguousarray(np.asarray(bq)[rows].reshape(2, 128).T),
            "bkc": np.ascontiguousarray(np.asarray(bk)[rows].reshape(2, 128).T),
            "bvr": np.ascontiguousarray(np.asarray(bv)[rows].reshape(1, D2)),
            "mask": mask,
            "onesr": ones,
            "zr": zr,
        })
    return in_maps


def kernel(X, Wq, bq, Wk, bk, Wv, bv, **kw):
    in_maps = _make_in_maps(X, Wq, bq, Wk, bk, Wv, bv)
    nc = _get_module()
    res = run_bass_kernel_spmd(nc, in_maps, core_ids=list(range(8)), **kw)
    _CACHE["last_res"] = res
    out = np.zeros((B, T, E), np.float32)
    for c in range(8):
        b, g = divmod(c, 4)
        out[b, :, D2 * g:D2 * g + D2] = res.results[c]["ot"].T
    return out


if __name__ == "__main__":
    _get_module()
    print("module built ok")

